# revision 34
# baseline (speedup 1.0000x reference)
"""Trainium2 Bass kernel for nn_EMHA (strided sparse attention block).

Math (per batch b of 4):
  XR = Wr @ x[b] + br                       (512, 4096)
  H  = raw view of XR as (4096, 512)        [free reshape in flat space]
  q/k/v = per-64-col-block H @ W{q,k,v}.T   (same 64x64 W for all 8 head-blocks)
  The (B,N,M,HD)->(B,N/S,M,S,HD) raw reshape + einsums reduce exactly to:
  32 independent attention groups (r = n%4, m = head): rows n==r (mod 4),
  cols [64m,64m+64), each a (1024 x 1024) softmax attention.
  OutMat (4096,512) viewed as (512,4096); out[b] = We @ OutMat_view + be.

Sharding: 8 cores = (b in 0..4) x (head-group hg in 0..2, 4 heads each).
A core only needs x / produces out columns n' with (n'%512)//256 == hg
(8 interleaved 256-wide stripes) -> no inter-core communication.

v3 changes vs baseline:
  - All matmul operands bf16 (same 1 cyc/row as f32r on TRN2 but ~6x
    faster LDWEIGHTS; halves input DMA bytes and SBUF traffic).
  - We-stage streams 512 cols for rr in {0,1} (OutMat columns regrouped
    (rr, du) so u and u+4 stripes are adjacent; host unshard permutes);
    rr in {2,3} stay 256-col du-halves so they can start earlier / the
    tail interleaves with the last AV.
  - Prologue: wrt + x(g=0) DMAs first; wet deferred into the filler
    queue; PE warm-up matmuls during the DMA wait (p-state ramp).
  - Filler gates retuned so rounds 6-7 keep a work reserve (V-blocks of
    stripes {2,6}/{3,7} deferred to gates 4/5, We du-halves at 6/7).
"""

import numpy as np

EMBED, M, S, HD = 1024, 8, 4, 64
B, N = 4, 4096
NCORES = 8

_SCALE = 1.0 / 32.0  # 1/sqrt(EMBED)

# OutMat 8-block column order is (rr, du) -> g = rr + 4*du
_G_ORDER = [0, 4, 1, 5, 2, 6, 3, 7]


def _build_nc(pack_e=True, repeat=1, loop_trips=0, filler_per_jb=None,
              av_jc=8, et_ic=2, warm_mm=14, defer_v=True, du_split=True,
              fp8dr=False, wet_gate=0):
    import contextlib
    import os as _os0
    _fpj = int(_os0.environ.get("K_FPJ", "-1"))

    import concourse.tile as tile
    from concourse import bacc, mybir

    dt = mybir.dt
    f32 = dt.float32
    bf16 = dt.bfloat16
    fp8 = dt.float8e4
    DR = mybir.MatmulPerfMode.DoubleRow

    import os as _os
    XINB = int(_os.environ.get("K_XINB", "2"))
    nc = bacc.Bacc(None, target_bir_lowering=False)

    xs = nc.dram_tensor("xs", [8, 1024, 256], bf16, kind="ExternalInput")
    wrt = nc.dram_tensor("wrt", [1024, 512], bf16, kind="ExternalInput")
    brb = nc.dram_tensor("brb", [128, 512], bf16, kind="ExternalInput")
    bdq = nc.dram_tensor("bdq", [128, 128], bf16, kind="ExternalInput")
    bdv = nc.dram_tensor("bdv", [128, 128], bf16, kind="ExternalInput")
    wet = nc.dram_tensor("wet", [512, 1024], bf16, kind="ExternalInput")
    beb = nc.dram_tensor("beb", [128, 8], f32, kind="ExternalInput")
    out = nc.dram_tensor("out", [1024, 2048], bf16, kind="ExternalOutput")

    with tile.TileContext(nc) as tc:
        with (
            tc.tile_pool(name="persist", bufs=1) as persist,
            tc.tile_pool(name="big", bufs=4) as bigpool,
            tc.tile_pool(name="xin", bufs=XINB) as xin,
            tc.tile_pool(name="outp", bufs=3) as outp,
            tc.tile_pool(name="small", bufs=4) as small,
            tc.tile_pool(name="ps512", bufs=2, space="PSUM") as ps512,
            tc.tile_pool(name="pse", bufs=2, space="PSUM") as pse,
            tc.tile_pool(name="ps128", bufs=2, space="PSUM") as ps128,
        ):
            wrt_sb = persist.tile([128, 8, 512], bf16, tag="wrt")
            brb_sb = persist.tile([128, 512], bf16, tag="brb")
            bdq_sb = persist.tile([128, 128], bf16, tag="bdq")
            bdv_sb = persist.tile([128, 128], bf16, tag="bdv")
            beb_sb = persist.tile([128, 8], f32, tag="beb")
            wet_sb = persist.tile([128, 4, 1024], bf16, tag="wet")

            if loop_trips > 1:
                rep_ctxs = [tc.For_i(0, loop_trips, 1)]
            else:
                rep_ctxs = [contextlib.nullcontext(None) for _ in range(repeat)]

            first_rep = True
            for _rep, _ctx in enumerate(rep_ctxs):
              with _ctx:
                qT = [persist.tile([128, 8, 512], bf16, tag=f"qT{p}",
                                   name=f"qT{_rep}_{p}") for p in range(2)]
                # kT is ht itself: the G-trick (bdq carries blockdiag(G),
                # G = Wq^T Wk) makes k = h raw, and ht's [token=(mb,d), u]
                # layout IS k^T for the two heads of pair p.
                ht_tiles = {}
                # V_sb[p][sig, sb, grp*65 + c]; col 64 of each 65-block = ones
                V_sb = [persist.tile([128, 32, 130], bf16, tag=f"V{p}",
                                     name=f"V{_rep}_{p}") for p in range(2)]
                OutMat = persist.tile([128, 4, 2048], bf16, tag="outmat",
                                      name=f"OutMat{_rep}")

                # ---------- stage-1/2 chunk emitters (filler steps) ----------
                def g_block_steps(g, dma_eng=None):
                    """Emit x-DMA now; return 8 filler closures (4 per pair)."""
                    eng = dma_eng or nc.sync
                    x_sb = xin.tile([128, 8, 256], bf16, tag="xin",
                                    name=f"x{_rep}_{g}")
                    # two strided DMAs (kc halves, matching the s1/s2 chunk
                    # split): 4x fewer SP descriptors than per-kc, and s1 can
                    # start after half the block lands
                    for h in range(2):
                        eng.dma_start(
                            x_sb[:, h * 4:(h + 1) * 4, :],
                            xs[g, h * 512:(h + 1) * 512, :]
                            .rearrange("(kc p) n -> p kc n", p=128))
                    steps = []
                    for p in range(2):
                        ht = [None]
                        acc = [None]

                        def s1(p=p, ht=ht, acc=acc, g=g):
                            ht[0] = persist.tile([128, 512], bf16,
                                                 tag=f"htg{g}_{p}",
                                                 name=f"ht{_rep}_{g}_{p}")
                            ht_tiles[(g, p)] = ht[0]
                            acc[0] = ps512.tile([128, 512], f32, tag="ps512",
                                                name=f"xacc{_rep}_{g}_{p}")
                            for kc in range(4):
                                nc.tensor.matmul(
                                    acc[0][:],
                                    x_sb[:, kc, p * 128:(p + 1) * 128],
                                    wrt_sb[:, kc, :],
                                    start=(kc == 0), stop=False)

                        def s2(p=p, ht=ht, acc=acc):
                            for kc in range(4, 8):
                                nc.tensor.matmul(
                                    acc[0][:],
                                    x_sb[:, kc, p * 128:(p + 1) * 128],
                                    wrt_sb[:, kc, :],
                                    start=False, stop=(kc == 7))
                            nc.vector.tensor_add(ht[0][:], acc[0][:], brb_sb[:])

                        def s3(p=p, ht=ht, g=g):
                            pq = ps512.tile([128, 512], f32, tag="ps512",
                                            name=f"pq{_rep}_{g}_{p}")
                            nc.tensor.matmul(pq[:], bdq_sb[:], ht[0][:],
                                             start=True, stop=True)
                            nc.vector.tensor_copy(out=qT[p][:, g, :], in_=pq[:])

                        def s4(p=p, ht=ht, g=g):
                            for sub in range(4):
                                sb = g * 4 + sub
                                pv = ps128.tile([128, 130], f32, tag="ps128",
                                                name=f"pv{_rep}_{g}_{p}_{sub}")
                                nc.tensor.matmul(
                                    pv[:, 0:128],
                                    ht[0][:, sub * 128:(sub + 1) * 128],
                                    bdv_sb[:],
                                    start=True, stop=True)
                                nc.vector.tensor_copy(
                                    out=V_sb[p][:, sb, :].rearrange(
                                        "q (gg c) -> q gg c", gg=2)[:, :, 0:64],
                                    in_=pv[:, 0:128].rearrange(
                                        "q (gg c) -> q gg c", gg=2))

                        steps += [s1, s2, s3, s4]
                    return steps

                def we_rr_steps(rr):
                    """512-col We matmuls for output cols [rr*512,(rr+1)*512)."""
                    steps = []
                    for ob in range(8):
                        def s(ob=ob, rr=rr):
                            pf = ps512.tile([128, 512], f32, tag="ps512",
                                            name=f"pf{_rep}_{rr}_{ob}")
                            for cc in range(4):
                                nc.tensor.matmul(
                                    pf[:],
                                    wet_sb[:, cc, ob * 128:(ob + 1) * 128],
                                    OutMat[:, cc, rr * 512:(rr + 1) * 512],
                                    start=(cc == 0), stop=(cc == 3))
                            ot = outp.tile([128, 512], bf16, tag="outp",
                                           name=f"ot{_rep}_{rr}_{ob}")
                            nc.vector.tensor_scalar_add(
                                out=ot[:], in0=pf[:],
                                scalar1=beb_sb[:, ob:ob + 1])
                            nc.sync.dma_start(
                                out[ob * 128:(ob + 1) * 128,
                                    rr * 512:(rr + 1) * 512], ot[:])
                        steps.append(s)
                    return steps

                def we_du_steps(rr, du, act_bias=False):
                    """256-col We matmuls for cols [rr*512+du*256, +256)."""
                    steps = []
                    c0 = rr * 512 + du * 256
                    for ob in range(8):
                        def s(ob=ob, c0=c0, rr=rr, du=du):
                            # tail steps alternate between the ps512 slots
                            # and the (now idle) energy-psum slots: 4-deep
                            # pf pipelining instead of 2
                            pool, ptag = ((pse, "pse") if act_bias and ob % 2
                                          else (ps512, "ps512"))
                            pf = pool.tile([128, 256], f32, tag=ptag,
                                           name=f"pfd{_rep}_{rr}_{du}_{ob}")
                            for cc in range(4):
                                nc.tensor.matmul(
                                    pf[:],
                                    wet_sb[:, cc, ob * 128:(ob + 1) * 128],
                                    OutMat[:, cc, c0:c0 + 256],
                                    start=(cc == 0), stop=(cc == 3))
                            ot = outp.tile([128, 256], bf16, tag="outp2",
                                           name=f"otd{_rep}_{rr}_{du}_{ob}")
                            if act_bias:
                                # Act engine is idle once the last exps are
                                # done; keeps the tail off the busy DVE
                                nc.scalar.add(ot[:], pf[:],
                                              beb_sb[:, ob:ob + 1])
                            else:
                                nc.vector.tensor_scalar_add(
                                    out=ot[:], in0=pf[:],
                                    scalar1=beb_sb[:, ob:ob + 1])
                            nc.sync.dma_start(
                                out[ob * 128:(ob + 1) * 128, c0:c0 + 256],
                                ot[:])
                        steps.append(s)
                    return steps

                # gated filler queue: (gate_t, closure); consumable when the
                # current round index t >= gate_t
                filler = []

                def drain_filler(t, budget=None):
                    n = 0
                    while filler and filler[0][0] <= t and (
                            budget is None or n < budget):
                        filler.pop(0)[1]()
                        n += 1

                def drain_adaptive(t, jb):
                    # spread eligible steps evenly over remaining jb slots
                    # (+4 virtual slots keep a reserve for the tail's
                    # exp-latency wait after round 7)
                    n_eligible = sum(1 for g_, _ in filler if g_ <= t)
                    slots = (8 - t) * 8 - jb + 4
                    budget = -(-n_eligible // max(1, slots))
                    drain_filler(t, budget=budget)

                def force_drain(gate):
                    # emit everything with gate_t <= gate (pops in queue
                    # order, skipping later-gated entries)
                    keep = []
                    while filler:
                        g_, s_ = filler.pop(0)
                        if g_ <= gate:
                            s_()
                        else:
                            keep.append((g_, s_))
                    filler.extend(keep)

                # ---------- attention round emitters ----------
                def emit_av_ib(t, ee, ib):
                    rr, p = t // 2, t % 2
                    po = ps128.tile([128, 130], f32, tag="ps128",
                                    name=f"po{_rep}_{t}_{ib}")
                    for grp in range(2):
                        for jc in range(av_jc):
                            sbj = 4 * rr + jc if jc < 4 else 4 * (rr + 4) + (jc - 4)
                            nc.tensor.matmul(
                                po[:, grp * 65:grp * 65 + 65],
                                ee[grp][:, jc, ib * 128:ib * 128 + 128],
                                V_sb[p][:, sbj, grp * 65:grp * 65 + 65],
                                start=(jc == 0), stop=(jc == av_jc - 1))
                    pov = po[:].rearrange("q (gg c) -> q gg c", gg=2)
                    rec = small.tile([128, 2], f32, tag="rec",
                                     name=f"rec{_rep}_{t}_{ib}")
                    nc.vector.reciprocal(out=rec[:], in_=pov[:, :, 64])
                    du = 0 if ib < 4 else 1
                    col = rr * 512 + du * 256 + p * 128
                    nc.vector.tensor_tensor(
                        OutMat[:, ib % 4, col:col + 128].rearrange(
                            "q (gg c) -> q gg c", gg=2),
                        pov[:, :, 0:64],
                        rec[:, :, None].to_broadcast((128, 2, 64)),
                        mybir.AluOpType.mult)

                def emit_round(t, prev_ee):
                    """E^T + exp for round t, with AV of t-1 and filler woven in."""
                    rr, p = t // 2, t % 2
                    ee = [bigpool.tile([128, 8, 1024], bf16, tag="big",
                                       name=f"ee{_rep}_{t}_{g_}")
                          for g_ in range(2)]
                    for jb in range(8):
                        gj = rr if jb < 4 else rr + 4
                        cj = (jb % 4) * 128
                        pe_t = [pse.tile([128, 1024], f32, tag="pse",
                                         name=f"pe{_rep}_{t}_{jb}_{g_}")
                                for g_ in range(2)]
                        for grp in range(2):
                            rows = slice(grp * 64, grp * 64 + 64)
                            if et_ic == 1:  # merged 1024-col energy matmul
                                kw = dict(start=True, stop=True)
                                if pack_e:
                                    kw["tile_position"] = (grp * 64, 0)
                                nc.tensor.matmul(
                                    pe_t[grp][:],
                                    ht_tiles[(gj, p)][rows, cj:cj + 128],
                                    qT[p][rows, rr:rr + 5:4, :],
                                    **kw)
                                continue
                            for ic in range(et_ic):
                                gi = rr if ic == 0 else rr + 4
                                kw = dict(start=True, stop=True)
                                if pack_e:
                                    kw["tile_position"] = (grp * 64, 0)
                                nc.tensor.matmul(
                                    pe_t[grp][:, ic * 512:(ic + 1) * 512],
                                    ht_tiles[(gj, p)][rows, cj:cj + 128],
                                    qT[p][rows, gi, :],
                                    **kw)
                        for grp in range(2):
                            nc.scalar.activation(
                                out=ee[grp][:, jb, :],
                                in_=pe_t[grp][:],
                                func=mybir.ActivationFunctionType.Exp,
                                scale=_SCALE)
                        if prev_ee is not None:
                            emit_av_ib(t - 1, prev_ee, jb)
                        if _fpj < 0:
                            drain_adaptive(t, jb)
                        else:
                            drain_filler(t, budget=_fpj)
                    return ee

                # ---------- prologue ----------
                # DMA priority: wrt chunks 0-3 + x(g=0) first (stage-1 deps),
                # then the rest; wet goes into the filler queue.
                # weights ride the Act HWDGE queue so they stream in
                # parallel with the x blocks on the SP queue (Act engine is
                # idle during the prologue)
                if first_rep:
                    nc.scalar.dma_start(
                        wrt_sb[:, 0:4, :],
                        wrt[0:512, :].rearrange("(kc p) u -> p kc u", p=128))
                st0 = g_block_steps(0)
                if first_rep:
                    nc.scalar.dma_start(
                        wrt_sb[:, 4:8, :],
                        wrt[512:1024, :].rearrange("(kc p) u -> p kc u",
                                                   p=128))
                    nc.scalar.dma_start(brb_sb[:], brb[:])
                    nc.scalar.dma_start(bdq_sb[:], bdq[:])
                    nc.scalar.dma_start(beb_sb[:], beb[:])
                st4 = g_block_steps(4)
                if first_rep:
                    nc.scalar.dma_start(bdv_sb[:], bdv[:])
                for p in range(2):
                    nc.vector.memset(V_sb[p][:, :, 64:65], 1.0)
                    nc.vector.memset(V_sb[p][:, :, 129:130], 1.0)

                # PE warm-up during the DMA wait: ramps the p-state so the
                # first real matmuls run at full clock. Depends only on a
                # memset tile; results are discarded.
                if first_rep and warm_mm > 0:
                    wsrc = persist.tile([128, 512], bf16, tag="warm")
                    nc.vector.memset(wsrc[:], 0.125)
                    wps = ps512.tile([128, 512], f32, tag="ps512",
                                     name="warmps")
                    for _w in range(warm_mm):
                        nc.tensor.matmul(wps[:], wsrc[:, 0:128], wsrc[:],
                                         start=True, stop=True,
                                         skip_group_check=True)
                first_rep = False

                deferred = []
                for i, s in enumerate(st0 + st4):
                    if i % 8 < 3:   # s1-s3 of p=0: round 0's q/k deps
                        s()
                    else:           # s4(p0) + all of p=1: needed from t=1
                        deferred.append((0, s))

                def wet_dma():
                    nc.sync.dma_start(
                        wet_sb[:, :, :],
                        wet.rearrange("(cc p) f -> p cc f", p=128))

                # stripe pairs {1,5}/{2,6}/{3,7} must be fully emitted before
                # rounds t=2/4/6 (their q/k feeds); V-blocks (s4) only before
                # the round that weaves their AV, so they are deferred further
                # to keep a PE work reserve for late rounds.
                filler.extend(deferred)
                filler.append((wet_gate, s) if False else (wet_gate, wet_dma))
                for g in (1, 5):
                    for s in g_block_steps(g):
                        filler.append((0, s))
                for g in (2, 6):
                    for i, s in enumerate(g_block_steps(g)):
                        filler.append(((4 if i % 4 == 3 else 1) if defer_v
                                       else 1, s))
                for g in (3, 7):
                    for i, s in enumerate(g_block_steps(g)):
                        filler.append(((5 if i % 4 == 3 else 3) if defer_v
                                       else 3, s))
                for s in we_rr_steps(0):
                    filler.append((3, s))
                for s in we_rr_steps(1):
                    filler.append((5, s))
                if du_split:
                    for s in we_du_steps(2, 0):
                        filler.append((6, s))
                    for s in we_du_steps(2, 1):
                        filler.append((7, s))
                else:
                    for s in we_rr_steps(2):
                        filler.append((7, s))
                    for s in we_du_steps(3, 0):
                        pass  # rr=3 emitted directly in the tail
                filler.sort(key=lambda e: e[0])

                prev = None
                for t in range(8):
                    if t == 1:
                        force_drain(0)  # V{0,4} needed by AV(0) woven here
                    elif t == 2:
                        force_drain(0)
                    elif t == 4:
                        force_drain(1)
                    elif t == 5:
                        force_drain(4)
                    elif t == 6:
                        force_drain(3)
                    elif t == 7:
                        force_drain(5)
                    prev = emit_round(t, prev)
                force_drain(7)
                for ib in range(4):
                    emit_av_ib(7, prev, ib)
                du0 = we_du_steps(3, 0, act_bias=True)
                for ib in range(4, 8):
                    emit_av_ib(7, prev, ib)
                    du0[2 * (ib - 4)]()
                    du0[2 * (ib - 4) + 1]()
                for s in we_du_steps(3, 1, act_bias=True):
                    s()

    nc.finalize()
    return nc


def _prep_inputs(x, Wq, Wk, Wv, Wr, br, We, be):
    import ml_dtypes
    bf16 = ml_dtypes.bfloat16

    x = np.asarray(x, np.float32)
    wrt = np.ascontiguousarray(np.asarray(Wr, np.float32).T.astype(bf16))
    wet = np.ascontiguousarray(np.asarray(We, np.float32).T.astype(bf16))
    brb = np.ascontiguousarray(
        np.broadcast_to(np.asarray(br, np.float32)[None, :],
                        (128, 512)).astype(bf16))
    beb = np.ascontiguousarray(np.asarray(be, np.float32).reshape(8, 128).T)

    def bd(w):
        z = np.zeros((128, 128), np.float32)
        wt = np.asarray(w, np.float32).T
        z[:64, :64] = wt
        z[64:, 64:] = wt
        return z.astype(bf16)

    # G-trick: E = (Wq h)·(Wk h) = (G^T h)·h with G = Wq^T Wk, so the
    # kernel projects q with blockdiag(G) and uses raw h as k.
    G = np.asarray(Wq, np.float32).T @ np.asarray(Wk, np.float32)
    bdq, bdv = bd(G.T), bd(Wv)
    shared = dict(wrt=wrt, wet=wet, brb=brb, beb=beb, bdq=bdq, bdv=bdv)
    in_maps = []
    for core in range(NCORES):
        b, hg = core // 2, core % 2
        xsh = np.ascontiguousarray(
            x[b].reshape(1024, 8, 2, 256)[:, :, hg, :]
            .transpose(1, 0, 2).astype(bf16))
        in_maps.append(dict(xs=xsh, **shared))
    return in_maps


def kernel(x, Wq, Wk, Wv, Wr, br, We, be, _trace=False, _pack_e=True):
    import os
    from concourse.bass_utils import run_bass_kernel_spmd

    nc = _build_nc(pack_e=_pack_e,
                   et_ic=int(os.environ.get("K_ETIC", "2")),
                   warm_mm=int(os.environ.get("K_WARM", "30")),
                   defer_v=os.environ.get("K_DEFV", "1") == "1",
                   du_split=os.environ.get("K_DUSPLIT", "1") == "1",
                   fp8dr=os.environ.get("K_FP8DR", "0") == "1",
                   wet_gate=int(os.environ.get("K_WETGATE", "0")))
    in_maps = _prep_inputs(x, Wq, Wk, Wv, Wr, br, We, be)
    res = run_bass_kernel_spmd(nc, in_maps, core_ids=list(range(NCORES)),
                               trace=_trace)
    outa = np.zeros((B, EMBED, N), np.float32)
    for core in range(NCORES):
        b, hg = core // 2, core % 2
        oc = np.asarray(res.results[core]["out"], np.float32)
        outa[b].reshape(1024, 8, 2, 256)[:, _G_ORDER, hg, :] = (
            oc.reshape(1024, 8, 256))
    if _trace:
        kernel._last_results = res
    return outa

